# revision 32
# baseline (speedup 1.0000x reference)
"""Trainium2 Bass kernel for nn_Decoder_5334349382400.

3-layer transformer decoder (self-attn + cross-attn + FFN + LN) with
norm-softmax pooling and a 2-class head, batch=1, seq 2048, hid 512.

Sharding: sequence-parallel over 8 NeuronCores (256 q-tokens/core),
with K/V computed redundantly per core from replicated inputs:
 - Every core receives the FULL trg and src. Cross-attention K/V
   (src-derived) and layer-0 self-attention K/V (trg-derived) are
   computed locally on every core - no gather needed.
 - After layers 0 and 1, the 2MB bf16 activation x is AllGathered
   once; each core then projects the full K/V for the next layer's
   self-attention itself (cheaper than gathering K+V, and the
   projection work doubles as PE filler during the gather).
 - Final pooling uses a tiny AllReduce of [wsum(512) | denom(1)].

Layout: activations live transposed in SBUF, xT[feat(part), tok(free)],
packed [128, 4, 256] (feat chunk-major). Attention scores are computed
transposed (scoresT[k, q]) so A@V needs no transpose; the softmax
denominator rides as a ones-column appended to V. Exp is batched
[128,1024] (one ScalarE activation per 4 key-tiles), and cross-attn
K/V projection matmuls are interleaved into the attention phases as
"side units" so the tensor engine never starves while ScalarE runs exp.
"""

import sys

sys.path.insert(0, "/opt/trn_rl_repo")

import numpy as np
import ml_dtypes

import concourse.bass as bass
import concourse.mybir as mybir
import concourse.tile as tile
from concourse import bacc, bass_utils

BF16 = ml_dtypes.bfloat16
F32 = mybir.dt.float32
BF = mybir.dt.bfloat16
AX = mybir.AxisListType
ALU = mybir.AluOpType
ACTF = mybir.ActivationFunctionType

C = 8          # cores
T = 2048       # tokens
TC = T // C    # tokens per core (256)
D = 512        # hidden
H = 8          # heads
HD = 64        # head dim
PF = 2048      # ffn dim
L = 3          # layers
ATOM = 64      # trg feature dim
NC4 = D // 128   # 4 feature chunks
NPF = PF // 128  # 16
NKT = T // 128   # 16 key tiles
EPS = 1e-5

# bias-pack column map
FT_B = 0
LBASE = 4
LSTRIDE = 44
SA_BQ, SA_BO, EA_BQ, EA_BO, B1, B2, LNG, LNB = 0, 4, 8, 12, 16, 32, 36, 40
FC1_B = LBASE + L * LSTRIDE          # 136
FC2_B = FC1_B + 2                    # 138
NCOL = FC2_B + 1                     # 139


def _bcol(l, off):
    return LBASE + l * LSTRIDE + off


def build_program():
    import os
    dbg_stages = bool(int(os.environ.get("KERNEL_DBG_STAGES", "0")))
    nc = bacc.Bacc("TRN2", target_bir_lowering=False, debug=False,
                   enable_asserts=True, num_devices=C)

    # ---- DRAM I/O ----
    t_trgT = nc.dram_tensor("trgT", [ATOM, T], BF, kind="ExternalInput")
    t_trgl = nc.dram_tensor("trgl", [ATOM, TC], BF, kind="ExternalInput")
    t_srcT = nc.dram_tensor("srcT", [D, T], BF, kind="ExternalInput")
    t_ftw = nc.dram_tensor("ftw", [ATOM, D], BF, kind="ExternalInput")
    t_bias = nc.dram_tensor("bias", [128, NCOL], F32, kind="ExternalInput")
    t_w = {}
    for l in range(L):
        for nm in ("saq", "sak", "sav", "sao", "eaq", "eak", "eav", "eao"):
            t_w[nm, l] = nc.dram_tensor(f"{nm}{l}", [D, D], BF, kind="ExternalInput")
        t_w["w1", l] = nc.dram_tensor(f"w1_{l}", [D, PF], BF, kind="ExternalInput")
        t_w["w2", l] = nc.dram_tensor(f"w2_{l}", [PF, D], BF, kind="ExternalInput")
    t_fc1 = nc.dram_tensor("fc1", [D, 256], BF, kind="ExternalInput")
    t_fc2 = nc.dram_tensor("fc2", [256, 2], BF, kind="ExternalInput")
    t_out = nc.dram_tensor("out", [1, 2], F32, kind="ExternalOutput")
    t_dbg = (nc.dram_tensor("dbg", [12 * 128, NC4 * TC], F32,
                            kind="ExternalOutput") if dbg_stages else None)

    rg = [list(range(C))]

    with tile.TileContext(nc) as tc:
        with (
            tc.tile_pool(name="dram", bufs=1, space="DRAM") as dram,
            tc.tile_pool(name="const", bufs=1) as cons,
            tc.tile_pool(name="state", bufs=1) as st,
            tc.tile_pool(name="wts", bufs=1) as wp,
            tc.tile_pool(name="wff", bufs=1) as wff,
            tc.tile_pool(name="small", bufs=1) as sm,
            tc.tile_pool(name="sma", bufs=2) as sma,
            tc.tile_pool(name="expp", bufs=2) as xp,
            tc.tile_pool(name="psS", bufs=2, space="PSUM") as psS,
            tc.tile_pool(name="psP", bufs=2, space="PSUM") as psP,
            tc.tile_pool(name="psO", bufs=1, space="PSUM") as psO,
            tc.tile_pool(name="psL", bufs=1, space="PSUM") as psL,
        ):
            # ---------- constants ----------
            bias_sb = cons.tile([128, NCOL], F32, tag="bias")
            nc.sync.dma_start(bias_sb[:], t_bias[:])
            ones_sb = cons.tile([128, 1], F32, tag="ones")
            nc.gpsimd.memset(ones_sb[:], 1.0)
            eps_sb = cons.tile([1, 1], F32, tag="eps")
            nc.gpsimd.memset(eps_sb[:], EPS)
            ftw_sb = cons.tile([ATOM, D], BF, tag="ftw")
            nc.sync.dma_start(ftw_sb[:], t_ftw[:])
            trgl_sb = cons.tile([ATOM, TC], BF, tag="trgl")
            nc.sync.dma_start(trgl_sb[:], t_trgl[:])

            # ---------- persistent state ----------
            xTf = st.tile([128, NC4, TC], F32, tag="xTf")     # x transposed, f32
            xTb = st.tile([128, NC4, TC], BF, tag="xTb")      # bf16 copy
            resid = st.tile([128, NC4, TC], F32, tag="resid")
            ffT = st.tile([128, NPF, TC], BF, tag="ffT")
            qT = st.tile([128, NC4, TC], BF, tag="qT")
            oT = st.tile([128, NC4, TC], BF, tag="oT")
            xfull = st.tile([128, NC4, T], BF, tag="xfull")   # gathered x (bf16)
            KTs = st.tile([128, NC4, T], BF, tag="KTs")
            Vs = st.tile([128, NKT, 8 * 65], BF, tag="Vs")
            KTe = st.tile([128, NC4, T], BF, tag="KTe")
            Ve = st.tile([128, NKT, 8 * 65], BF, tag="Ve")

            # trg^T staged in the (not yet used) ffT tile
            trgT_ap = ffT[0:ATOM, 0:8, :]
            nc.sync.dma_start(
                trgT_ap, t_trgT.ap().rearrange("p (a b) -> p a b", b=TC))

            # ones columns for the softmax denominator (written once;
            # V evacuations never touch the 65th column of each head)
            nc.gpsimd.memset(
                Vs[:].rearrange("p k (h e) -> p k h e", e=65)[:, :, :, 64:65], 1.0)
            nc.gpsimd.memset(
                Ve[:].rearrange("p k (h e) -> p k h e", e=65)[:, :, :, 64:65], 1.0)

            def dump(stage, tile_f32):
                if t_dbg is not None:
                    nc.sync.dma_start(
                        t_dbg.ap()[128 * stage:128 * stage + 128, :],
                        tile_f32[:].rearrange("p a b -> p (a b)"))

            def bcolap(col):
                return bias_sb[:, col:col + 1]

            def hrows(tl, h):
                """head h rows of a feat-packed [128, NC4, X] tile -> [64, X]."""
                o = 64 * (h % 2)
                return tl[o:o + 64, h // 2, :]

            def wload(tag, dram_t):
                w = wp.tile([128, NC4, D], BF, tag=tag)
                nc.sync.dma_start(
                    w[:], dram_t.ap().rearrange("(c p) o -> p c o", p=128))
                return w

            # -------- side-work machinery (PE filler units) --------
            SIDE = []

            def pull(n=1):
                for _ in range(n):
                    if SIDE:
                        SIDE.pop(0)()

            def kv_units(KT, Vg, wk_sb, wv_sb, xsrc):
                """Full-sequence K/V projection for one attention, split into
                ~860ns matmul units. Returns (main, late): `late` holds the
                K chunks 2,3 (only read by attention heads 4-7), safe to run
                as that same attention's first side units."""

                def vu(kt):
                    ps = psP.tile([128, 512], F32, tag="psp")
                    for c in range(NC4):
                        nc.tensor.matmul(
                            ps[:], lhsT=xsrc[:, c, 128 * kt:128 * kt + 128],
                            rhs=wv_sb[:, c, :],
                            start=(c == 0), stop=(c == NC4 - 1))
                    dst = Vg[:, kt, :].rearrange("p (h e) -> p h e", e=65)[:, :, 0:64]
                    nc.vector.tensor_copy(
                        dst, ps[:].rearrange("p (h e) -> p h e", e=64))

                def ku(m, tb):
                    ps = psP.tile([128, 512], F32, tag="psp")
                    for c in range(NC4):
                        nc.tensor.matmul(
                            ps[:], lhsT=wk_sb[:, c, 128 * m:128 * m + 128],
                            rhs=xsrc[:, c, 512 * tb:512 * tb + 512],
                            start=(c == 0), stop=(c == NC4 - 1))
                    nc.vector.tensor_copy(KT[:, m, 512 * tb:512 * tb + 512], ps[:])

                main = [lambda kt=kt: vu(kt) for kt in range(NKT)]
                main += [lambda m=m, tb=tb: ku(m, tb)
                         for m in range(2) for tb in range(4)]
                late = [lambda m=m, tb=tb: ku(m, tb)
                        for m in range(2, NC4) for tb in range(4)]
                return main, late

            # -------- building blocks --------
            def proj_loc(out_sb, w_sb, bias_col=None):
                """[512,512] projection over the local 256 tokens."""
                for m in range(NC4):
                    ps = psP.tile([128, 512], F32, tag="psp")
                    for c in range(NC4):
                        nc.tensor.matmul(
                            ps[:, 0:TC], lhsT=w_sb[:, c, 128 * m:128 * m + 128],
                            rhs=xTb[:, c, :],
                            start=(c == 0), stop=(c == NC4 - 1))
                    if bias_col is not None:
                        nc.vector.tensor_scalar_add(
                            out_sb[:, m, :], ps[:, 0:TC], bcolap(bias_col + m))
                    else:
                        nc.vector.tensor_copy(out_sb[:, m, :], ps[:, 0:TC])

            def attention(KT, Vg, wq_sb, wo_sb, bq_col, bo_col, after_q=None,
                          own=None):
                own = list(own) if own else []

                def pull1():
                    if own:
                        own.pop(0)()
                    elif SIDE:
                        SIDE.pop(0)()

                proj_loc(qT, wq_sb, bias_col=bq_col)
                if after_q is not None:
                    after_q()
                for hp in range(H // 2):
                    pso = psO.tile([65, 512], F32, tag="psoT")
                    for h2 in range(2):
                        h = 2 * hp + h2
                        col = 256 * h2
                        exs = []

                        def avg(g):
                            ex = exs[g]
                            for j in range(4):
                                kt = 4 * g + j
                                nc.tensor.matmul(
                                    pso[:, col:col + TC],
                                    lhsT=Vg[:, kt, 65 * h:65 * h + 65],
                                    rhs=ex[:, j, :],
                                    start=(kt == 0), stop=(kt == NKT - 1))

                        for g in range(4):
                            ps = psS.tile([128, 4, TC], F32, tag="scoresT")
                            for j in range(4):
                                kt = 4 * g + j
                                nc.tensor.matmul(
                                    ps[:, j, :],
                                    lhsT=hrows(KT, h)[:, 128 * kt:128 * kt + 128],
                                    rhs=hrows(qT, h),
                                    start=True, stop=True)
                            ex = xp.tile([128, 4, TC], BF, tag="expT")
                            nc.scalar.activation(ex[:], ps[:], ACTF.Exp,
                                                 scale=float(1.0 / np.sqrt(HD)))
                            exs.append(ex)
                            if g >= 1:
                                pull1()
                                avg(g - 1)
                        pull1()
                        avg(3)
                    # normalize the head pair (recip via SBUF: the custom DVE
                    # approx op reads garbage from PSUM on HW)
                    dcp = sm.tile([1, 512], F32, tag="dcp")
                    nc.vector.tensor_copy(dcp[:], pso[64:65, :])
                    den = sm.tile([1, 512], F32, tag="rden")
                    nc.vector.reciprocal_approx_fast(den[:], dcp[:])
                    rb = sm.tile([64, 512], F32, tag="rbh")
                    nc.gpsimd.partition_broadcast(rb[:], den[:])
                    for h2 in range(2):
                        h = 2 * hp + h2
                        nc.vector.tensor_tensor(
                            hrows(oT, h), pso[0:64, 256 * h2:256 * h2 + TC],
                            rb[:, 256 * h2:256 * h2 + TC], op=ALU.mult)
                for u in own:  # safety drain (normally consumed by pull1)
                    u()
                # attn out projection + bias + residual -> resid (f32)
                for m in range(NC4):
                    ps = psP.tile([128, 512], F32, tag="psp")
                    for c in range(NC4):
                        nc.tensor.matmul(ps[:, 0:TC],
                                         lhsT=wo_sb[:, c, 128 * m:128 * m + 128],
                                         rhs=oT[:, c, :],
                                         start=(c == 0), stop=(c == NC4 - 1))
                    nc.vector.scalar_tensor_tensor(
                        resid[:, m, :], ps[:, 0:TC], bcolap(bo_col + m),
                        xTf[:, m, :], op0=ALU.add, op1=ALU.add)

            def sum_sq_rows(psl, src):
                """psl[0:1] += col-sums(src), psl[32:33] += col-sums(src^2)."""
                for c in range(NC4):
                    nc.tensor.matmul(psl[0:1, :], lhsT=ones_sb[:], rhs=src[:, c, :],
                                     start=(c == 0), stop=(c == NC4 - 1))
                for c in range(NC4):
                    s1 = sma.tile([128, TC], F32, tag="sqc")
                    nc.vector.tensor_tensor(s1[:], src[:, c, :], src[:, c, :],
                                            op=ALU.mult)
                    nc.tensor.matmul(psl[32:33, :], lhsT=ones_sb[:], rhs=s1[:],
                                     start=(c == 0), stop=(c == NC4 - 1))

            def layer_norm(gcol, bcol):
                """resid[128,NC4,TC] f32 -> xTf, xTb (feat-dim LN in T layout)."""
                psl = psL.tile([33, TC], F32, tag="psl")
                sum_sq_rows(psl, resid)
                mn = sm.tile([1, TC], F32, tag="mn")
                nc.scalar.mul(mn[:], psl[0:1, :], 1.0 / D)
                m2 = sm.tile([1, TC], F32, tag="m2")
                nc.vector.tensor_tensor(m2[:], mn[:], mn[:], op=ALU.mult)
                ve = sm.tile([1, TC], F32, tag="ve")
                nc.vector.scalar_tensor_tensor(ve[:], psl[32:33, :], 1.0 / D, m2[:],
                                               op0=ALU.mult, op1=ALU.subtract)
                # rsqrt via ln/exp - keeps ScalarE on one ACT table set
                # (Sqrt lives in a different set; each switch costs ~2.7us)
                lnv = sm.tile([1, TC], F32, tag="vs")
                nc.scalar.activation(lnv[:], ve[:], ACTF.Ln, bias=eps_sb[:])
                rs = sm.tile([1, TC], F32, tag="rs")
                nc.scalar.activation(rs[:], lnv[:], ACTF.Exp, scale=-0.5)
                mb = sm.tile([128, TC], F32, tag="mb")
                nc.gpsimd.partition_broadcast(mb[:], mn[:])
                rb = sm.tile([128, TC], F32, tag="rsb")
                nc.gpsimd.partition_broadcast(rb[:], rs[:])
                for m in range(NC4):
                    t1 = sm.tile([128, TC], F32, tag="t1")
                    nc.vector.tensor_tensor(t1[:], resid[:, m, :], mb[:],
                                            op=ALU.subtract)
                    t2 = sma.tile([128, TC], F32, tag="t2")
                    nc.vector.tensor_tensor(t2[:], t1[:], rb[:], op=ALU.mult)
                    nc.scalar.activation(xTf[:, m, :], t2[:], ACTF.Identity,
                                         bias=bcolap(bcol + m), scale=bcolap(gcol + m))
                    nc.scalar.activation(xTb[:, m, :], t2[:], ACTF.Identity,
                                         bias=bcolap(bcol + m), scale=bcolap(gcol + m))

            # ================= program =================
            # weights for layer 0 front
            sak = wload("sak", t_w["sak", 0])
            sav = wload("sav", t_w["sav", 0])
            wq = wload("wq", t_w["saq", 0])
            srcT_sb = cons.tile([128, NC4, T], BF, tag="srcT")
            nc.sync.dma_start(srcT_sb[:],
                              t_srcT.ap().rearrange("(c p) t -> p c t", p=128))
            eak = wload("eak", t_w["eak", 0])
            eav = wload("eav", t_w["eav", 0])
            wo = wload("wo", t_w["sao", 0])

            # x0 full (for layer-0 SA K/V): xfull = ftw^T @ trgT + ft_b
            for m in range(NC4):
                for tb in range(4):
                    ps = psP.tile([128, 512], F32, tag="psp")
                    nc.tensor.matmul(ps[:],
                                     lhsT=ftw_sb[:, 128 * m:128 * m + 128],
                                     rhs=trgT_ap[:, 2 * tb:2 * tb + 2, :],
                                     start=True, stop=True)
                    nc.scalar.activation(xfull[:, m, 512 * tb:512 * tb + 512], ps[:],
                                         ACTF.Identity, bias=bcolap(FT_B + m))
            # x0 local shard
            for m in range(NC4):
                ps = psP.tile([128, 512], F32, tag="psp")
                nc.tensor.matmul(ps[:, 0:TC], lhsT=ftw_sb[:, 128 * m:128 * m + 128],
                                 rhs=trgl_sb[:], start=True, stop=True)
                nc.scalar.activation(xTf[:, m, :], ps[:, 0:TC], ACTF.Identity,
                                     bias=bcolap(FT_B + m))
                nc.scalar.activation(xTb[:, m, :], ps[:, 0:TC], ACTF.Identity,
                                     bias=bcolap(FT_B + m))

            dump(0, xTf)

            # SA0 K/V (inline, from locally computed xfull); K chunks 2,3
            # deferred into the SA0 attention as its own side units
            sa_main, sa_late = kv_units(KTs, Vs, sak, sav, xfull)
            for u in sa_main:
                u()

            # rest of layer-0 weights
            w1_sb = wff.tile([128, NC4, PF], BF, tag="w1")
            nc.sync.dma_start(w1_sb[:],
                              t_w["w1", 0].ap().rearrange("(c p) o -> p c o", p=128))
            w2_sb = wff.tile([128, NPF, D], BF, tag="w2")
            nc.sync.dma_start(w2_sb[:],
                              t_w["w2", 0].ap().rearrange("(c p) o -> p c o", p=128))
            fc1_sb = cons.tile([128, NC4, 256], BF, tag="fc1")
            nc.sync.dma_start(fc1_sb[:], t_fc1.ap().rearrange("(c p) o -> p c o", p=128))
            fc2_sb = cons.tile([128, 2, 2], BF, tag="fc2")
            nc.sync.dma_start(fc2_sb[:], t_fc2.ap().rearrange("(c p) o -> p c o", p=128))

            WNEXT = {}

            for l in range(L):
                # ---- self attention (side: EA K/V main of this layer) ----
                if l == 0:
                    ea_main, ea_late = kv_units(KTe, Ve, eak, eav, srcT_sb)
                    SIDE.extend(ea_main)

                def load_eaq(l=l):
                    WNEXT['wq'] = wload("wq", t_w["eaq", l])

                attention(KTs, Vs, wq, wo, _bcol(l, SA_BQ), _bcol(l, SA_BO),
                          after_q=load_eaq, own=sa_late)
                wq = WNEXT['wq']
                wo = wload("wo", t_w["eao", l])
                pull(max(0, len(SIDE) - 4))  # flush most EA main units
                if l == 0:
                    dump(10, resid)
                layer_norm(_bcol(l, LNG), _bcol(l, LNB))
                pull(len(SIDE))  # the rest keep PE warm through the LN tail
                dump(1 + 3 * l, xTf)

                # prefetch next layer's SA/EA K,V weights (tags now WAR-free)
                if l + 1 < L:
                    sak = wload("sak", t_w["sak", l + 1])
                    sav = wload("sav", t_w["sav", l + 1])
                    eak_n = wload("eak", t_w["eak", l + 1])
                    eav_n = wload("eav", t_w["eav", l + 1])

                # ---- cross attention ----
                def load_saq(l=l):
                    if l + 1 < L:
                        WNEXT['wq'] = wload("wq", t_w["saq", l + 1])

                attention(KTe, Ve, wq, wo, _bcol(l, EA_BQ), _bcol(l, EA_BO),
                          after_q=load_saq, own=ea_late)
                if l + 1 < L:
                    wq = WNEXT['wq']
                    wo = wload("wo", t_w["sao", l + 1])
                    # next layer's EA K/V units become side work for the
                    # LN2/FFN/LN3 tails and the gather wait
                    eak, eav = eak_n, eav_n
                    ea_main, ea_late = kv_units(KTe, Ve, eak, eav, srcT_sb)
                    SIDE.extend(ea_main)
                layer_norm(_bcol(l, LNG), _bcol(l, LNB))
                pull(4)
                dump(2 + 3 * l, xTf)

                # ---- FFN ----
                for m in range(NPF):
                    ps = psP.tile([128, 512], F32, tag="psp")
                    for c in range(NC4):
                        nc.tensor.matmul(ps[:, 0:TC],
                                         lhsT=w1_sb[:, c, 128 * m:128 * m + 128],
                                         rhs=xTb[:, c, :],
                                         start=(c == 0), stop=(c == NC4 - 1))
                    nc.scalar.activation(ffT[:, m, :], ps[:, 0:TC], ACTF.Relu,
                                         bias=bcolap(_bcol(l, B1) + m))
                    if m in (7, 15):
                        pull(2)
                for m in range(NC4):
                    ps = psP.tile([128, 512], F32, tag="psp")
                    for c in range(NPF):
                        nc.tensor.matmul(ps[:, 0:TC],
                                         lhsT=w2_sb[:, c, 128 * m:128 * m + 128],
                                         rhs=ffT[:, c, :],
                                         start=(c == 0), stop=(c == NPF - 1))
                    nc.vector.scalar_tensor_tensor(
                        resid[:, m, :], ps[:, 0:TC], bcolap(_bcol(l, B2) + m),
                        xTf[:, m, :], op0=ALU.add, op1=ALU.add)
                layer_norm(_bcol(l, LNG), _bcol(l, LNB))
                pull(4)
                dump(3 + 3 * l, xTf)

                if l + 1 < L:
                    # ---- AllGather x for next layer's SA ----
                    agi = dram.tile([D, TC], BF, tag=f"agi{l}")
                    ago = dram.tile([C * D, TC], BF, tag=f"ago{l}",
                                    addr_space="Shared")
                    nc.sync.dma_start(
                        agi[:].rearrange("(c p) t -> p c t", p=128), xTb[:])
                    nc.gpsimd.collective_compute(
                        "AllGather", ALU.bypass, replica_groups=rg,
                        ins=[agi[:]], outs=[ago[:]])
                    # load gathered x, project next SA K/V
                    src_x = ago[:].rearrange("(r c p) t -> c p r t", p=128, c=NC4)
                    for c in range(NC4):
                        nc.sync.dma_start(
                            xfull[:, c, :].rearrange("p (r t) -> p r t", t=TC),
                            src_x[c])
                    # FFN weights for next layer (w1/w2 tags now WAR-free)
                    nc.sync.dma_start(
                        w1_sb[:],
                        t_w["w1", l + 1].ap().rearrange("(c p) o -> p c o", p=128))
                    nc.sync.dma_start(
                        w2_sb[:],
                        t_w["w2", l + 1].ap().rearrange("(c p) o -> p c o", p=128))
                    pull(8)  # remaining EA units fill the gather wait
                    sa_main, sa_late = kv_units(KTs, Vs, sak, sav, xfull)
                    for u in sa_main:
                        u()

            # ---- pooling: softmax over token norms, then weighted sum ----
            psl = psL.tile([33, TC], F32, tag="psl")
            sum_sq_rows(psl, xTf)
            lnn = sm.tile([1, TC], F32, tag="lnn")
            nc.scalar.activation(lnn[:], psl[32:33, :], ACTF.Ln)
            nrm = sm.tile([1, TC], F32, tag="nrm")
            nc.scalar.activation(nrm[:], lnn[:], ACTF.Exp, scale=0.5)
            ew = sm.tile([1, TC], F32, tag="ew")
            nc.scalar.activation(ew[:], nrm[:], ACTF.Exp)
            denl = sm.tile([1, 1], F32, tag="denl")
            nc.vector.reduce_sum(denl[:], ew[:], axis=AX.X)
            ewb = sm.tile([128, TC], F32, tag="ewb")
            nc.gpsimd.partition_broadcast(ewb[:], ew[:])
            ws = sm.tile([128, NC4 + 1], F32, tag="ws")
            for m in range(NC4):
                t1 = sma.tile([128, TC], F32, tag="t1")
                nc.vector.tensor_tensor(t1[:], xTf[:, m, :], ewb[:], op=ALU.mult)
                nc.vector.reduce_sum(ws[:, m:m + 1], t1[:], axis=AX.X)
            nc.vector.tensor_copy(ws[0:1, NC4:NC4 + 1], denl[:])

            ar_in = dram.tile([513, 1], F32, tag="ar_in")
            ar_out = dram.tile([513, 1], F32, tag="ar_out")
            nc.sync.dma_start(
                ar_in[0:512, :].rearrange("(c p) o -> p (c o)", p=128),
                ws[:, 0:NC4])
            nc.sync.dma_start(ar_in[512:513, :], ws[0:1, NC4:NC4 + 1])
            nc.gpsimd.collective_compute("AllReduce", ALU.add, replica_groups=rg,
                                         ins=[ar_in[:]], outs=[ar_out[:]])

            wsg = sm.tile([128, NC4], F32, tag="wsg")
            nc.sync.dma_start(
                wsg[:], ar_out[0:512, :].rearrange("(c p) o -> p (c o)", p=128))
            deng = sm.tile([1, 1], F32, tag="deng")
            nc.sync.dma_start(deng[:], ar_out[512:513, :])
            rd = sm.tile([1, 1], F32, tag="rd")
            nc.vector.reciprocal(rd[:], deng[:])
            rdb = sm.tile([128, 1], F32, tag="rdb")
            nc.gpsimd.partition_broadcast(rdb[:], rd[:])
            pooledT = sm.tile([128, NC4], BF, tag="pooledT")
            nc.vector.tensor_scalar_mul(pooledT[:], wsg[:], rdb[:])

            h1T = sm.tile([128, 2, 1], BF, tag="h1T")
            for m in range(2):
                ps = psP.tile([128, 512], F32, tag="psp")
                for c in range(NC4):
                    nc.tensor.matmul(ps[:, 0:1],
                                     lhsT=fc1_sb[:, c, 128 * m:128 * m + 128],
                                     rhs=pooledT[:, c:c + 1],
                                     start=(c == 0), stop=(c == NC4 - 1))
                nc.scalar.activation(h1T[:, m, :], ps[:, 0:1], ACTF.Relu,
                                     bias=bcolap(FC1_B + m))
            ps2 = psP.tile([128, 512], F32, tag="psp")
            for c in range(2):
                nc.tensor.matmul(ps2[0:2, 0:1], lhsT=fc2_sb[:, c, :],
                                 rhs=h1T[:, c, :],
                                 start=(c == 0), stop=(c == 1))
            lab = sm.tile([2, 1], F32, tag="lab")
            nc.scalar.activation(lab[:], ps2[0:2, 0:1], ACTF.Identity,
                                 bias=bias_sb[0:2, FC2_B:FC2_B + 1])
            nc.sync.dma_start(t_out.ap().rearrange("a b -> b a"), lab[:])

    nc.compile()
    return nc


_PROGRAM = None


def _get_program():
    global _PROGRAM
    if _PROGRAM is None:
        _PROGRAM = build_program()
    return _PROGRAM


def _host_inputs(inputs):
    f = {k: np.asarray(v, np.float32) for k, v in inputs.items()}

    def bf(x):
        return np.ascontiguousarray(np.asarray(x, np.float32).astype(BF16))

    bias = np.zeros((128, NCOL), np.float32)

    def put(col, vec):
        v = np.asarray(vec, np.float32).reshape(-1)
        for c in range(len(v) // 128):
            bias[:, col + c] = v[128 * c:128 * c + 128]

    put(FT_B, f['ft_b'])
    for l in range(L):
        put(_bcol(l, SA_BQ), f['sa_bq'][l])
        put(_bcol(l, SA_BO), f['sa_bv'][l] @ f['sa_wo'][l] + f['sa_bo'][l])
        put(_bcol(l, EA_BQ), f['ea_bq'][l])
        put(_bcol(l, EA_BO), f['ea_bv'][l] @ f['ea_wo'][l] + f['ea_bo'][l])
        put(_bcol(l, B1), f['pf_b1'][l])
        put(_bcol(l, B2), f['pf_b2'][l])
        put(_bcol(l, LNG), f['ln_g'][l])
        put(_bcol(l, LNB), f['ln_b'][l])
    put(FC1_B, f['fc1_b'])
    bias[0:2, FC2_B] = f['fc2_b']

    shared = {'ftw': bf(f['ft_w']), 'bias': bias,
              'fc1': bf(f['fc1_w']), 'fc2': bf(f['fc2_w']),
              'trgT': bf(f['trg'][0].T), 'srcT': bf(f['src'][0].T)}
    for l in range(L):
        shared[f'saq{l}'] = bf(f['sa_wq'][l])
        shared[f'sak{l}'] = bf(f['sa_wk'][l])
        shared[f'sav{l}'] = bf(f['sa_wv'][l])
        shared[f'sao{l}'] = bf(f['sa_wo'][l])
        shared[f'eaq{l}'] = bf(f['ea_wq'][l])
        shared[f'eak{l}'] = bf(f['ea_wk'][l])
        shared[f'eav{l}'] = bf(f['ea_wv'][l])
        shared[f'eao{l}'] = bf(f['ea_wo'][l])
        shared[f'w1_{l}'] = bf(f['pf_w1'][l])
        shared[f'w2_{l}'] = bf(f['pf_w2'][l])

    in_maps = []
    for i in range(C):
        sl = slice(TC * i, TC * (i + 1))
        m = dict(shared)
        m['trgl'] = bf(f['trg'][0, sl, :].T)
        in_maps.append(m)
    return in_maps


def kernel(**inputs):
    import os
    nc = _get_program()
    in_maps = _host_inputs(inputs)
    trace = bool(int(os.environ.get("KERNEL_TRACE", "0")))
    res = bass_utils.run_bass_kernel_spmd(
        nc, in_maps, core_ids=list(range(C)), trace=trace)
    if trace:
        kernel.last_exec_time_ns = res.exec_time_ns
        kernel.last_results = res
    return np.asarray(res.results[0]["out"], np.float32)
